# revision 1
# baseline (speedup 1.0000x reference)
"""GQA transformer block on 8 TRN2 cores.

Sharding: core = (b, k) with b = batch (2), k = kv-head (4).
Each core computes LN1(tokens[b]), projects Q (its 4 q-heads), K/V (its kv
head), does causal attention in S^T orientation (keys on partitions, queries
on the free dim) with unnormalized exp-scores plus a ones-column to produce
softmax denominators from the same matmul, then multiplies by its Wo row-slab
to get a partial [N, D] output. A 4-core ReduceScatter sums the partials and
hands each core a 512-row slab for residual + LN2.

All heavy matmuls run as float32r (1 cycle/row when free >= 256).
"""

import os
import sys
from contextlib import ExitStack

for _p in ("/opt/trn_rl_repo", "/root/.axon_site/_ro/trn_rl_repo"):
    if os.path.isdir(_p) and _p not in sys.path:
        sys.path.insert(0, _p)

import numpy as np

import concourse.bass as bass
import concourse.bacc as bacc
import concourse.tile as tile
from concourse import mybir
from concourse.bass_utils import run_bass_kernel_spmd
from concourse.masks import make_identity

B, N, D = 2, 2048, 1024
HQ, HKV, HD = 16, 4, 64
G = HQ // HKV  # q heads per kv head (= per core)
EPS = 1e-5
P = 128
NT = N // P  # 16 token tiles
DC = D // P  # 8 d-chunks
SLAB = N // 4  # 512 rows per core after reduce-scatter
ST = SLAB // P  # 4 token tiles per slab
F32 = mybir.dt.float32
F32R = mybir.dt.float32r
FD = 512  # matmul free-dim chunk (one PSUM bank)
NC_ = N // FD  # 4 free chunks over queries
RG = [[0, 1, 2, 3], [4, 5, 6, 7]]
AF = mybir.ActivationFunctionType
ALU = mybir.AluOpType
SCALE = 1.0 / np.sqrt(HD)


def _ln_stats(nc, pool, x_ap, eps_tile, p=P):
    """mean/rstd over the free dim (len 1024) of x_ap [p, 1024]."""
    stats = pool.tile([p, 2, nc.vector.BN_STATS_DIM], F32, tag="bst")
    xg = x_ap.rearrange("p (s f) -> p s f", s=2)
    for s in range(2):
        nc.vector.bn_stats(out=stats[:, s, :], in_=xg[:, s, :])
    mv = pool.tile([p, nc.vector.BN_AGGR_DIM], F32, tag="mv")
    nc.vector.bn_aggr(out=mv[:, :], in_=stats[:, :, :])
    rstd = pool.tile([p, 1], F32, tag="rstd")
    nc.scalar.activation(out=rstd[:, :], in_=mv[:, 1:2], func=AF.Sqrt,
                         bias=eps_tile[:p, :], scale=1.0)
    nc.vector.reciprocal(out=rstd[:, :], in_=rstd[:, :])
    return mv, rstd


def build_program():
    nc = bacc.Bacc(None, target_bir_lowering=False, num_devices=8)
    x = nc.declare_dram_parameter("x", [N, D], F32, isOutput=False)
    xs = nc.declare_dram_parameter("xs", [SLAB, D], F32, isOutput=False)
    wq = nc.declare_dram_parameter("wq", [D, G * HD], F32R, isOutput=False)
    wkv = nc.declare_dram_parameter("wkv", [D, 2 * HD], F32R, isOutput=False)
    wo = nc.declare_dram_parameter("wo", [G * HD, D], F32R, isOutput=False)
    g1b1 = nc.declare_dram_parameter("g1b1", [2, D], F32, isOutput=False)
    g2b2 = nc.declare_dram_parameter("g2b2", [2, D], F32, isOutput=False)
    mask = nc.declare_dram_parameter("mask", [P, P], F32, isOutput=False)
    y = nc.declare_dram_parameter("y", [SLAB, D], F32, isOutput=True)

    with tile.TileContext(nc) as tc, ExitStack() as ctx:
        const = ctx.enter_context(tc.tile_pool(name="const", bufs=1))
        big = ctx.enter_context(tc.tile_pool(name="big", bufs=1))
        work = ctx.enter_context(tc.tile_pool(name="work", bufs=3))
        stp = ctx.enter_context(tc.tile_pool(name="stats", bufs=4))
        outp = ctx.enter_context(tc.tile_pool(name="outp", bufs=3))
        pss = ctx.enter_context(tc.tile_pool(name="pss", bufs=4, space="PSUM"))
        pso = ctx.enter_context(tc.tile_pool(name="pso", bufs=1, space="PSUM"))
        dram = ctx.enter_context(tc.tile_pool(name="dram", bufs=1, space="DRAM"))

        # ---- constants ----
        ident = const.tile([P, P], F32)
        make_identity(nc, ident)
        identr = const.tile([P, P], F32R)
        nc.scalar.copy(out=identr[:, :], in_=ident[:, :])
        mask_sb = const.tile([P, P], F32)
        nc.sync.dma_start(out=mask_sb[:, :], in_=mask[:, :])
        eps_t = const.tile([P, 1], F32)
        nc.vector.memset(eps_t[:, :], EPS)
        ones01f = const.tile([1, HD], F32)
        nc.vector.memset(ones01f[:, :], 0.1)
        ones01 = const.tile([1, HD], F32R)
        nc.scalar.copy(out=ones01[:, :], in_=ones01f[:, :])  # 0.1 residual scale
        g1c = const.tile([P, DC], F32)
        b1c = const.tile([P, DC], F32)
        nc.sync.dma_start(out=g1c[:, :], in_=g1b1[0, :].rearrange("(c p) -> p c", p=P))
        nc.sync.dma_start(out=b1c[:, :], in_=g1b1[1, :].rearrange("(c p) -> p c", p=P))
        g2bc = const.tile([P, D], F32)
        b2bc = const.tile([P, D], F32)
        for row, dst in ((0, g2bc), (1, b2bc)):
            src = g2b2[row : row + 1, :]
            bsrc = bass.AP(tensor=src.tensor, offset=src.offset,
                           ap=[[0, P]] + src.ap[1:])
            nc.sync.dma_start(out=dst[:, :], in_=bsrc)

        # weights
        wq_sb = const.tile([P, DC, G * HD], F32R)
        nc.sync.dma_start(out=wq_sb[:, :, :],
                          in_=wq.rearrange("(c p) m -> p c m", p=P))
        wkv_sb = const.tile([P, DC, 2 * HD], F32R)
        nc.sync.dma_start(out=wkv_sb[:, :, :],
                          in_=wkv.rearrange("(c p) m -> p c m", p=P))
        wo_sb = const.tile([P, 2, D], F32R)
        nc.sync.dma_start(out=wo_sb[:, :, :],
                          in_=wo.rearrange("(m p) d -> p m d", p=P))

        # ---- big persistent tensors ----
        tpool = ctx.enter_context(tc.tile_pool(name="tpool", bufs=2))
        qT = big.tile([HD, G, N], F32R)      # Q^T per head, all at base partition 0
        kvT = big.tile([P, N], F32R)         # rows 0-63 K^T, 64-127 V^T
        vt1 = big.tile([P, NT, HD + 1], F32R)  # [V_j | ones] per key chunk
        attnT = big.tile([P, 2, N], F32R)    # normalized attn^T (256 x N)

        # ---- stage 1+2: LN1 + transpose + projections, per 512-token chunk ----
        for f in range(NC_):
            tT = tpool.tile([P, DC, FD], F32R, tag="tT")
            for it in range(FD // P):
                i = f * (FD // P) + it
                xt = work.tile([P, D], F32, tag="xt")
                nc.sync.dma_start(out=xt[:, :], in_=x[i * P : (i + 1) * P, :])
                mv, rstd = _ln_stats(nc, stp, xt[:, :], eps_t)
                nc.vector.tensor_scalar(out=xt[:, :], in0=xt[:, :],
                                        scalar1=mv[:, 0:1], scalar2=rstd[:, :],
                                        op0=ALU.subtract, op1=ALU.mult)
                for c in range(DC):
                    pt = pss.tile([P, FD], F32, tag="ps")
                    nc.tensor.transpose(pt[:, :P], xt[:, c * P : (c + 1) * P],
                                        ident[:, :])
                    nc.vector.tensor_scalar(out=tT[:, c, it * P : (it + 1) * P],
                                            in0=pt[:, :P],
                                            scalar1=g1c[:, c : c + 1],
                                            scalar2=b1c[:, c : c + 1],
                                            op0=ALU.mult, op1=ALU.add)
            # Q^T: psum rows 0-63 = head 2m, 64-127 = head 2m+1
            for m in range(2):
                ps = pss.tile([P, FD], F32, tag="ps")
                for c in range(DC):
                    nc.tensor.matmul(ps[:, :],
                                     wq_sb[:, c, m * P : (m + 1) * P],
                                     tT[:, c, :],
                                     start=(c == 0), stop=(c == DC - 1))
                nc.scalar.copy(out=qT[:, 2 * m, f * FD : (f + 1) * FD],
                               in_=ps[:HD, :])
                # upper half must land at base partition 0 -> SBUF bounce + DMA
                qtmp = work.tile([P, FD], F32R, tag="qtmp")
                nc.scalar.copy(out=qtmp[HD:P, :], in_=ps[HD:P, :])
                nc.sync.dma_start(out=qT[:, 2 * m + 1, f * FD : (f + 1) * FD],
                                  in_=qtmp[HD:P, :])
            ps = pss.tile([P, FD], F32, tag="ps")
            for c in range(DC):
                nc.tensor.matmul(ps[:, :], wkv_sb[:, c, :], tT[:, c, :],
                                 start=(c == 0), stop=(c == DC - 1))
            nc.scalar.copy(out=kvT[:, f * FD : (f + 1) * FD], in_=ps[:, :])

        # V^T -> V (per key chunk), plus the ones column
        onescol = const.tile([P, 1], F32)
        nc.vector.memset(onescol[:, :], 1.0)
        for j in range(NT):
            nc.scalar.copy(out=vt1[:, j, HD : HD + 1], in_=onescol[:, :])
            pt = pss.tile([P, FD], F32R, tag="ps")
            nc.tensor.transpose(pt[:, :HD], kvT[HD:P, j * P : (j + 1) * P],
                                identr[HD:P, HD:P])
            nc.scalar.copy(out=vt1[:, j, :HD], in_=pt[:, :HD])

        # ---- stage 3: attention, S^T orientation ----
        for h in range(G):
            qrow = (h % 2) * HD  # target row range inside attnT chunk h // 2
            qm = h // 2
            psO = pso.tile([HD + 1, N], F32)
            for c in range(NC_):
                c0 = c * FD
                for j in range(4 * c + 4):
                    q0 = max(c0, j * P)
                    w = (c + 1) * FD - q0
                    psS = pss.tile([P, FD], F32, tag="ps")
                    nc.tensor.matmul(psS[:, :w],
                                     kvT[0:HD, j * P : (j + 1) * P],
                                     qT[:, h, q0 : q0 + w],
                                     start=True, stop=True)
                    uT = work.tile([P, FD], F32R, tag="ut")
                    nc.scalar.activation(out=uT[:, :w], in_=psS[:, :w],
                                         func=AF.Exp, scale=SCALE)
                    if j // 4 == c:  # diagonal block -> causal mask
                        o = j * P - c0
                        nc.vector.tensor_mul(uT[:, o : o + P], uT[:, o : o + P],
                                             mask_sb[:, :])
                    nc.tensor.matmul(psO[:, q0 : q0 + w], vt1[:, j, :],
                                     uT[:, :w],
                                     start=(j == 0), stop=(j == 4 * c + 3))
                # normalize: attnT = psO[0:64] * (0.1 / Z)
                rz = stp.tile([1, FD], F32R, tag="rz")
                with nc.allow_low_precision(reason="1/Z in f32r feeds f32r matmul"):
                    nc.vector.reciprocal(out=rz[:, :],
                                         in_=psO[HD : HD + 1, c0 : c0 + FD])
                psB = pss.tile([P, FD], F32, tag="ps")
                nc.tensor.matmul(psB[:HD, :], ones01[:, :], rz[:, :],
                                 start=True, stop=True)
                bz = work.tile([HD, FD], F32, tag="bz")
                nc.scalar.copy(out=bz[:, :], in_=psB[:HD, :])
                if qrow == 0:
                    nc.vector.tensor_mul(attnT[0:HD, qm, c0 : c0 + FD],
                                         psO[0:HD, c0 : c0 + FD], bz[:, :])
                else:
                    at = work.tile([HD, FD], F32R, tag="atmp")
                    nc.vector.tensor_mul(at[:, :], psO[0:HD, c0 : c0 + FD],
                                         bz[:, :])
                    nc.sync.dma_start(out=attnT[HD:P, qm, c0 : c0 + FD],
                                      in_=at[:, :])

        # ---- stage 4: Wo partial product -> DRAM ----
        part = dram.tile([N, D], F32)
        rs = dram.tile([SLAB, D], F32)
        for i in range(NT):
            op = outp.tile([P, D], F32, tag="op")
            for f in range(2):
                ps = pss.tile([P, FD], F32, tag="ps")
                for m in range(2):
                    nc.tensor.matmul(ps[:, :],
                                     attnT[:, m, i * P : (i + 1) * P],
                                     wo_sb[:, m, f * FD : (f + 1) * FD],
                                     start=(m == 0), stop=(m == 1))
                nc.scalar.copy(out=op[:, f * FD : (f + 1) * FD], in_=ps[:, :])
            nc.sync.dma_start(out=part[i * P : (i + 1) * P, :], in_=op[:, :])

        # ---- stage 5: reduce-scatter + residual + LN2 ----
        nc.gpsimd.collective_compute(
            "ReduceScatter", ALU.add, replica_groups=RG,
            ins=[part[:, :]], outs=[rs[:, :]],
        )
        for t in range(ST):
            rt = work.tile([P, D], F32, tag="xt")
            nc.sync.dma_start(out=rt[:, :], in_=rs[t * P : (t + 1) * P, :])
            xt = work.tile([P, D], F32, tag="xt")
            nc.sync.dma_start(out=xt[:, :], in_=xs[t * P : (t + 1) * P, :])
            nc.vector.tensor_add(rt[:, :], rt[:, :], xt[:, :])
            mv, rstd = _ln_stats(nc, stp, rt[:, :], eps_t)
            nc.vector.tensor_scalar(out=rt[:, :], in0=rt[:, :],
                                    scalar1=mv[:, 0:1], scalar2=rstd[:, :],
                                    op0=ALU.subtract, op1=ALU.mult)
            nc.vector.tensor_mul(rt[:, :], rt[:, :], g2bc[:, :])
            nc.vector.tensor_add(rt[:, :], rt[:, :], b2bc[:, :])
            nc.sync.dma_start(out=y[t * P : (t + 1) * P, :], in_=rt[:, :])

    nc.finalize()
    return nc


_NC_CACHE = {}


def _get_program():
    if "nc" not in _NC_CACHE:
        _NC_CACHE["nc"] = build_program()
    return _NC_CACHE["nc"]


def make_in_maps(tokens, Wq, Wk, Wv, Wo, g1, b1, g2, b2):
    tokens = np.ascontiguousarray(tokens, np.float32)
    mask = np.triu(np.ones((P, P), np.float32))  # [key, query]: key <= query
    g1b1 = np.stack([np.asarray(g1, np.float32), np.asarray(b1, np.float32)])
    g2b2 = np.stack([np.asarray(g2, np.float32), np.asarray(b2, np.float32)])
    in_maps = []
    for cid in range(8):
        b, k = cid // 4, cid % 4
        r = cid % 4
        in_maps.append({
            "x": tokens[b],
            "xs": tokens[b][r * SLAB : (r + 1) * SLAB],
            "wq": np.ascontiguousarray(Wq[:, k * G * HD : (k + 1) * G * HD], np.float32),
            "wkv": np.ascontiguousarray(
                np.concatenate([Wk[:, k * HD : (k + 1) * HD],
                                Wv[:, k * HD : (k + 1) * HD]], axis=1), np.float32),
            "wo": np.ascontiguousarray(Wo[k * G * HD : (k + 1) * G * HD, :], np.float32),
            "g1b1": g1b1, "g2b2": g2b2, "mask": mask,
        })
    return in_maps


def kernel(tokens, Wq, Wk, Wv, Wo, g1, b1, g2, b2, _trace=False, _trace_kwargs=None):
    nc = _get_program()
    in_maps = make_in_maps(tokens, Wq, Wk, Wv, Wo, g1, b1, g2, b2)
    res = run_bass_kernel_spmd(nc, in_maps, list(range(8)),
                               trace=_trace, **(_trace_kwargs or {}))
    out = np.empty((B, N, D), np.float32)
    for cid in range(8):
        b, r = cid // 4, cid % 4
        out[b, r * SLAB : (r + 1) * SLAB] = res.results[cid]["y"]
    if _trace:
        return out, res
    return out

